# revision 1
# baseline (speedup 1.0000x reference)
"""AutoCorrelationLayer kernel for 8 TRN2 NeuronCores.

Math (per reference): Q/K/V projections (D=2048, H=8 heads, DH=256),
circular cross-correlation along the head dim per (b,h,l) implemented as
half-spectrum DFT matmuls (L==S==DH==256, real inputs -> conjugate-symmetric
spectrum, and the DC bin is a per-row constant that softmax ignores, so
frequencies 1..128 suffice), softmax over the correlation axis, time-delay
aggregation (per-(b,h) 256x256 matmul with V), output projection.

Distribution: pure data-parallel over batch (B=32 -> 4 batches/core, zero
collectives).  All compute in fp16 operands with fp32 PSUM accumulation
(validated ~2.8e-3 rel err vs fp32 reference).  Activations are staged
feature-major (contraction dim on partitions) via host-side transposes of the
input shards; weights are passed transposed for the same reason.
"""

import numpy as np

import concourse.bass as bass
import concourse.mybir as mybir
import concourse.tile as tile_mod
from concourse.tile import TileContext
from concourse.vector_clock import ScopedClock
from concourse.bass_utils import run_bass_kernel_spmd

F32 = mybir.dt.float32
F16 = mybir.dt.float16
AF = mybir.ActivationFunctionType
AX = mybir.AxisListType

B, L, D, H = 32, 256, 2048, 8
DH = D // H          # 256
NCORES = 8
BPC = B // NCORES    # 4 batches per core
T = BPC * L          # 1024 tokens per core
NHALF = 2
TH = T // NHALF      # 512 tokens per half
EC = D // 128        # 16 feature chunks
DC = D // 128        # 16 contraction chunks
NF = 128             # retained spectrum bins (freqs 1..128)


def _patch_tile_drain():
    """This walrus build allows at most ONE semaphore wait per instruction;
    Tile's kernel-tail drain collects one wait per live semaphore on a single
    Drain.  Split the extras onto additional drain instructions."""
    if getattr(tile_mod.TileContext, "_drain_split_patched", False):
        return

    def _drain_and_barrier(self, tick_clock, wait_clock):
        nc = self.nc
        drain_inst = nc.sync.drain()
        wait_clock.add_sem_waits(
            drain_inst.ins, ScopedClock({None: tick_clock.global_clock})
        )
        si = drain_inst.ins.sync_info
        waits = list(si.on_wait) if si is not None and si.on_wait else []
        if len(waits) > 1:
            drain_inst.ins.sync_info = mybir.SyncInfo(
                on_wait=[waits[0]], on_update=list(si.on_update or [])
            )
            for w in waits[1:]:
                extra = nc.sync.drain()
                extra.ins.sync_info = mybir.SyncInfo(on_wait=[w], on_update=[])
        nc.all_engine_barrier()
        popped = nc._tile_sem_poison_stack.pop()
        assert popped is self._sem_poison
        nc.clear_and_free_semaphores(list(self.sems.allocated().values()))
        nc.all_engine_barrier()

    tile_mod.TileContext._drain_and_barrier = _drain_and_barrier
    tile_mod.TileContext._drain_split_patched = True


def _split_multiwaits(nc):
    """Walrus in this build rejects >1 semaphore wait per instruction.  Hoist
    extra waits onto standalone EventSemaphore NOPs inserted just before the
    offending instruction on the same engine (engines execute in order)."""
    uid = [0]
    for fn in nc.m.functions:
        for bb in fn.blocks:
            il = bb.instructions
            i = 0
            while i < len(il):
                inst = il[i]
                si = inst.sync_info
                waits = list(si.on_wait) if si is not None and si.on_wait else []
                if len(waits) > 1:
                    carriers = []
                    for w in waits[:-1]:
                        uid[0] += 1
                        es = mybir.InstEventSemaphore(
                            name=f"mwsplit_{uid[0]}",
                            engine=inst.engine,
                            ins=[], outs=[],
                            sync_info=mybir.SyncInfo(on_wait=[w], on_update=[]),
                        )
                        carriers.append(es)
                    inst.sync_info = mybir.SyncInfo(
                        on_wait=[waits[-1]], on_update=list(si.on_update or [])
                    )
                    il[i:i] = carriers
                    i += len(carriers)
                i += 1


def build_kernel():
    _patch_tile_drain()
    nc = bass.Bass()

    xq = nc.declare_dram_parameter("xq", [D, T], F32, isOutput=False)  # queries^T
    xk = nc.declare_dram_parameter("xk", [D, T], F32, isOutput=False)
    xv = nc.declare_dram_parameter("xv", [D, T], F32, isOutput=False)
    wq = nc.declare_dram_parameter("wq", [D, D], F32, isOutput=False)  # Wq^T [d,e]
    wk = nc.declare_dram_parameter("wk", [D, D], F32, isOutput=False)
    wv = nc.declare_dram_parameter("wv", [D, D], F32, isOutput=False)
    wo = nc.declare_dram_parameter("wo", [D, D], F32, isOutput=False)
    bq = nc.declare_dram_parameter("bq", [D], F32, isOutput=False)
    bk = nc.declare_dram_parameter("bk", [D], F32, isOutput=False)
    bv = nc.declare_dram_parameter("bv", [D], F32, isOutput=False)
    bo = nc.declare_dram_parameter("bo", [D], F32, isOutput=False)
    tmp = nc.declare_dram_parameter("temp", [H], F32, isOutput=False)
    dfwd = nc.declare_dram_parameter("dfwd", [2, DH, NF], F32, isOutput=False)
    dinv = nc.declare_dram_parameter("dinv", [2, NF, DH], F32, isOutput=False)
    idn = nc.declare_dram_parameter("idn", [128, 128], F32, isOutput=False)
    out = nc.declare_dram_parameter("out", [T, D], F32, isOutput=True)

    def bcast_ap(param, n):
        return bass.AP(tensor=param, offset=0, ap=[[0, 128], [1, n]])

    with TileContext(nc) as tc:
        import contextlib

        with contextlib.ExitStack() as ctx:
            consts = ctx.enter_context(tc.tile_pool(name="consts", bufs=1))
            stg = ctx.enter_context(tc.tile_pool(name="stg", bufs=4))
            wstg = ctx.enter_context(tc.tile_pool(name="wstg", bufs=6))
            persist = ctx.enter_context(tc.tile_pool(name="persist", bufs=1))
            small = ctx.enter_context(tc.tile_pool(name="small", bufs=16))

            # ---- constants ----
            ident16 = consts.tile([128, 128], F16)
            s = stg.tile([128, 128], F32, tag="stg_c")
            nc.sync.dma_start(out=s, in_=idn[:])
            nc.vector.tensor_copy(ident16, s)

            # fwd DFT mats [m%128, mc, f=128]; inv mats [f, n=256]
            fmats = []
            for i in range(2):
                t16 = consts.tile([128, 2, NF], F16, name=f"dfwd{i}", tag=f"dfwd{i}")
                for c in range(2):
                    sd = stg.tile([128, NF], F32, tag="stg_c")
                    nc.sync.dma_start(out=sd, in_=dfwd[i, c * 128:(c + 1) * 128, :])
                    nc.vector.tensor_copy(t16[:, c, :], sd)
                fmats.append(t16)
            C_sb, S_sb = fmats
            imats = []
            for i in range(2):
                t16i = consts.tile([128, DH], F16, name=f"dinv{i}", tag=f"dinv{i}")
                sd2 = stg.tile([128, DH], F32, tag="stg_c")
                nc.sync.dma_start(out=sd2, in_=dinv[i, :, :])
                nc.vector.tensor_copy(t16i, sd2)
                imats.append(t16i)
            Ci_sb, Si_sb = imats

            # biases as per-partition columns [128, EC]
            bq_sb = consts.tile([128, EC], F32)
            bk_sb = consts.tile([128, EC], F32)
            bv_sb = consts.tile([128, EC], F32)
            for bsb, bpar in ((bq_sb, bq), (bk_sb, bk), (bv_sb, bv)):
                nc.sync.dma_start(out=bsb, in_=bpar[:].rearrange("(ec p) -> p ec", p=128))
            # bo broadcast across partitions [128, D] and 1/temp columns
            bo_bc = consts.tile([128, D], F32)
            nc.sync.dma_start(out=bo_bc, in_=bcast_ap(bo, D))
            temp_bc = consts.tile([128, H], F32)
            nc.sync.dma_start(out=temp_bc, in_=bcast_ap(tmp, H))
            tinv = consts.tile([128, H], F32)
            nc.vector.reciprocal(tinv, temp_bc)
            ntinv = consts.tile([128, H], F32)
            nc.vector.tensor_scalar_mul(ntinv, tinv, -1.0)

            outf16 = persist.tile([128, EC, T], F16)      # Out_f^T [e, t]

            with tc.tile_pool(name="qkpool", bufs=1) as qkpool:
                q16 = qkpool.tile([128, EC, T], F16, tag="q16")
                k16 = qkpool.tile([128, EC, T], F16, tag="k16")

                # ---------- Q/K projections over full T (weights streamed once) ----
                for (dst16, xpar, wpar, bsb) in ((q16, xq, wq, bq_sb), (k16, xk, wk, bk_sb)):
                    with tc.tile_pool(name="xqk", bufs=1) as xqk, \
                         tc.tile_pool(name="psP", bufs=8, space="PSUM") as psP:
                        x16 = xqk.tile([128, DC, T], F16, tag="x16b")
                        pss0 = [psP.tile([128, TH], F32, tag="ps_proj",
                                         name=f"psp_0_{jt}") for jt in range(8)]
                        for dc in range(DC):
                            sw = wstg.tile([128, 512], F32, tag="stg_w")
                            nc.sync.dma_start(out=sw, in_=wpar[dc * 128:(dc + 1) * 128, 0:512])
                            for tn in range(2):
                                sx = stg.tile([128, TH], F32, tag="stg_x")
                                nc.sync.dma_start(
                                    out=sx,
                                    in_=xpar[dc * 128:(dc + 1) * 128, tn * TH:(tn + 1) * TH])
                                if dc % 2 == 0:
                                    nc.vector.tensor_copy(x16[:, dc, tn * TH:(tn + 1) * TH], sx)
                                else:
                                    nc.scalar.activation(x16[:, dc, tn * TH:(tn + 1) * TH],
                                                         sx, AF.Identity)
                            w16 = wstg.tile([128, 512], F16, tag="w16")
                            nc.vector.tensor_copy(w16, sw)
                            for j in range(4):
                                for tn in range(2):
                                    nc.tensor.matmul(
                                        pss0[j * 2 + tn], w16[:, j * 128:(j + 1) * 128],
                                        x16[:, dc, tn * TH:(tn + 1) * TH],
                                        start=(dc == 0), stop=(dc == DC - 1))
                        for ecg in range(EC // 4):
                            pss = pss0 if ecg == 0 else [
                                psP.tile([128, TH], F32, tag="ps_proj",
                                         name=f"psp_{ecg}_{jt}") for jt in range(8)]
                            if ecg > 0:
                                for dc in range(DC):
                                    sw = wstg.tile([128, 512], F32, tag="stg_w")
                                    nc.sync.dma_start(
                                        out=sw,
                                        in_=wpar[dc * 128:(dc + 1) * 128, ecg * 512:(ecg + 1) * 512])
                                    w16 = wstg.tile([128, 512], F16, tag="w16")
                                    nc.vector.tensor_copy(w16, sw)
                                    for j in range(4):
                                        for tn in range(2):
                                            nc.tensor.matmul(
                                                pss[j * 2 + tn], w16[:, j * 128:(j + 1) * 128],
                                                x16[:, dc, tn * TH:(tn + 1) * TH],
                                                start=(dc == 0), stop=(dc == DC - 1))
                            for j in range(4):
                                ec = ecg * 4 + j
                                for tn in range(2):
                                    if tn == 0:
                                        nc.scalar.activation(
                                            dst16[:, ec, tn * TH:(tn + 1) * TH],
                                            pss[j * 2 + tn], AF.Identity,
                                            bias=bsb[:, ec:ec + 1])
                                    else:
                                        nc.vector.tensor_scalar_add(
                                            dst16[:, ec, tn * TH:(tn + 1) * TH],
                                            pss[j * 2 + tn], bsb[:, ec:ec + 1])

                with tc.tile_pool(name="vpool", bufs=1) as vpool:
                    v16 = vpool.tile([128, TH // 128, D], F16)  # token-major V (per half)

                    for half in range(NHALF):
                        t0 = half * TH

                        # ---------- V projection, token-major (no transposes) ------
                        with tc.tile_pool(name="xvpool", bufs=1) as xvpool, \
                             tc.tile_pool(name="psV", bufs=6, space="PSUM") as psV:
                            xv16 = xvpool.tile([128, DC, TH], F16, tag="xv16")
                            for dc in range(DC):
                                sx = stg.tile([128, TH], F32, tag="stg_x")
                                nc.sync.dma_start(
                                    out=sx, in_=xv[dc * 128:(dc + 1) * 128, t0:t0 + TH])
                                if dc % 2 == 0:
                                    nc.vector.tensor_copy(xv16[:, dc, :], sx)
                                else:
                                    nc.scalar.activation(xv16[:, dc, :], sx, AF.Identity)
                            for ecg in range(EC // 4):
                                psv = [psV.tile([128, 512], F32, tag="ps_vproj",
                                                name=f"psv_{half}_{ecg}_{tck}")
                                       for tck in range(4)]
                                for dc in range(DC):
                                    sw = wstg.tile([128, 512], F32, tag="stg_w")
                                    nc.sync.dma_start(
                                        out=sw,
                                        in_=wv[dc * 128:(dc + 1) * 128, ecg * 512:(ecg + 1) * 512])
                                    w16 = wstg.tile([128, 512], F16, tag="w16")
                                    nc.vector.tensor_copy(w16, sw)
                                    for tck in range(4):
                                        nc.tensor.matmul(
                                            psv[tck], xv16[:, dc, tck * 128:(tck + 1) * 128],
                                            w16[:],
                                            start=(dc == 0), stop=(dc == DC - 1))
                                for tck in range(4):
                                    nc.vector.tensor_copy(
                                        v16[:, tck, ecg * 512:(ecg + 1) * 512], psv[tck])

                        # ---------- per-head spectrum corr + softmax + TDA ---------
                        with tc.tile_pool(name="hpool", bufs=2) as hpool, \
                             tc.tile_pool(name="epool", bufs=6) as epool, \
                             tc.tile_pool(name="psD", bufs=2, space="PSUM") as psD, \
                             tc.tile_pool(name="psB", bufs=3, space="PSUM") as psB, \
                             tc.tile_pool(name="psT", bufs=1, space="PSUM") as psT, \
                             tc.tile_pool(name="psO", bufs=2, space="PSUM") as psO:
                            for h in range(H):
                                qr = hpool.tile([128, TH], F16, tag="qr")
                                qi = hpool.tile([128, TH], F16, tag="qi")
                                kr = hpool.tile([128, TH], F16, tag="kr")
                                ki = hpool.tile([128, TH], F16, tag="ki")
                                for dst, src16, mat in ((qr, q16, C_sb), (qi, q16, S_sb),
                                                        (kr, k16, C_sb), (ki, k16, S_sb)):
                                    ps = psD.tile([128, TH], F32, tag="ps_dft")
                                    for mc in range(2):
                                        nc.tensor.matmul(
                                            ps, mat[:, mc, :],
                                            src16[:, h * 2 + mc, t0:t0 + TH],
                                            start=(mc == 0), stop=(mc == 1))
                                    nc.vector.tensor_copy(dst, ps)
                                pr = hpool.tile([128, TH], F16, tag="pr")
                                pi = hpool.tile([128, TH], F16, tag="pi")
                                tmp16 = hpool.tile([128, TH], F16, tag="tmp16")
                                nc.vector.tensor_mul(pr, qr, kr)
                                nc.vector.tensor_mul(tmp16, qi, ki)
                                nc.vector.tensor_add(pr, pr, tmp16)
                                nc.vector.tensor_mul(pi, qi, kr)
                                nc.vector.tensor_mul(tmp16, qr, ki)
                                nc.vector.tensor_sub(pi, pi, tmp16)

                                et16 = hpool.tile([128, 2, TH], F16, tag="et16")
                                for tck in range(TH // 128):
                                    psc = psB.tile([128, DH], F32, tag="ps_corr")
                                    nc.tensor.matmul(psc, pr[:, tck * 128:(tck + 1) * 128],
                                                     Ci_sb[:], start=True, stop=False)
                                    nc.tensor.matmul(psc, pi[:, tck * 128:(tck + 1) * 128],
                                                     Si_sb[:], start=False, stop=True)
                                    mx = small.tile([128, 1], F32, tag="mx")
                                    nc.vector.reduce_max(mx, psc[:], axis=AX.X)
                                    nbias = small.tile([128, 1], F32, tag="nbias")
                                    nc.vector.tensor_scalar_mul(nbias, mx, ntinv[:, h:h + 1])
                                    e16 = epool.tile([128, DH], F16, tag="e16")
                                    ssum = small.tile([128, 1], F32, tag="ssum")
                                    nc.scalar.activation(e16, psc[:], AF.Exp,
                                                         bias=nbias[:], scale=tinv[:, h:h + 1],
                                                         accum_out=ssum[:])
                                    rinv = small.tile([128, 1], F32, tag="rinv")
                                    nc.vector.reciprocal(rinv, ssum)
                                    en16 = epool.tile([128, DH], F16, tag="en16")
                                    nc.scalar.activation(en16, e16, AF.Identity, scale=rinv[:])
                                    for sc in range(2):
                                        pst = psT.tile([128, 128], F16, tag="ps_et")
                                        nc.tensor.transpose(
                                            pst, en16[:, sc * 128:(sc + 1) * 128], ident16[:])
                                        nc.vector.tensor_copy(
                                            et16[:, sc, tck * 128:(tck + 1) * 128], pst)
                                # TDA: Outf^T[i, t] += Vp[s,i].T @ E^T[s,t] per local batch
                                for b in range(TH // L):
                                    for ic in range(2):
                                        pso = psO.tile([128, L], F32, tag="ps_tda")
                                        for sc in range(2):
                                            nc.tensor.matmul(
                                                pso,
                                                v16[:, b * 2 + sc,
                                                    h * DH + ic * 128:h * DH + (ic + 1) * 128],
                                                et16[:, sc, b * L:(b + 1) * L],
                                                start=(sc == 0), stop=(sc == 1))
                                        nc.scalar.activation(
                                            outf16[:, h * 2 + ic, t0 + b * L:t0 + (b + 1) * L],
                                            pso, AF.Identity,
                                            bias=bv_sb[:, h * 2 + ic:h * 2 + ic + 1])

            # ---------- output projection: Y[t,o] = Outf^T.T @ Wo^T + bo ----------
            with tc.tile_pool(name="wopool", bufs=1) as wopool, \
                 tc.tile_pool(name="ypool", bufs=4) as ypool, \
                 tc.tile_pool(name="psY", bufs=8, space="PSUM") as psY:
                wo16 = wopool.tile([128, EC, D], F16)
                for ec in range(EC):
                    sw = wopool.tile([128, D], F32, tag="stg_wo", bufs=3, name=f"stg_wo_{ec}")
                    nc.sync.dma_start(out=sw, in_=wo[ec * 128:(ec + 1) * 128, :])
                    if ec % 2 == 0:
                        nc.vector.tensor_copy(wo16[:, ec, :], sw)
                    else:
                        nc.scalar.activation(wo16[:, ec, :], sw, AF.Identity)
                for tck in range(T // 128):
                    pss = [psY.tile([128, 512], F32, tag="ps_y", name=f"ps_y_{tck}_{i}")
                           for i in range(4)]
                    for ec in range(EC):
                        for oc in range(4):
                            nc.tensor.matmul(pss[oc], outf16[:, ec, tck * 128:(tck + 1) * 128],
                                             wo16[:, ec, oc * 512:(oc + 1) * 512],
                                             start=(ec == 0), stop=(ec == EC - 1))
                    for oc in range(4):
                        yt = ypool.tile([128, 512], F32, tag="yt")
                        nc.vector.tensor_add(yt, pss[oc], bo_bc[:, oc * 512:(oc + 1) * 512])
                        nc.sync.dma_start(out=out[tck * 128:(tck + 1) * 128, oc * 512:(oc + 1) * 512],
                                          in_=yt)
    _split_multiwaits(nc)
    return nc


_NC_CACHE = None


def _get_nc():
    global _NC_CACHE
    if _NC_CACHE is None:
        _NC_CACHE = build_kernel()
    return _NC_CACHE


def _dft_consts():
    m = np.arange(DH, dtype=np.float64)
    f = np.arange(1, NF + 1, dtype=np.float64)   # freqs 1..128 (DC dropped: softmax-invariant)
    ang_f = 2.0 * np.pi * np.outer(m, f) / DH
    C = np.cos(ang_f)            # [m, NF]
    S = -np.sin(ang_f)
    n = np.arange(DH, dtype=np.float64)
    w = np.where(f < NF, 2.0, 1.0)[:, None]      # conjugate-symmetry weights; Nyquist = 1
    ang_i = 2.0 * np.pi * np.outer(f, n) / DH
    Ci = w * np.cos(ang_i) / DH  # [NF, n]
    Si = -w * np.sin(ang_i) / DH
    dfwd = np.stack([C, S]).astype(np.float32)
    dinv = np.stack([Ci, Si]).astype(np.float32)
    return dfwd, dinv


def make_in_maps(inputs):
    dfwd, dinv = _dft_consts()
    idn = np.eye(128, dtype=np.float32)
    shared = {
        "wq": np.ascontiguousarray(inputs["Wq"].T).astype(np.float32, copy=False),
        "wk": np.ascontiguousarray(inputs["Wk"].T).astype(np.float32, copy=False),
        "wv": np.ascontiguousarray(inputs["Wv"].T).astype(np.float32, copy=False),
        "wo": np.ascontiguousarray(inputs["Wo"].T).astype(np.float32, copy=False),
        "bq": np.asarray(inputs["bq"], np.float32),
        "bk": np.asarray(inputs["bk"], np.float32),
        "bv": np.asarray(inputs["bv"], np.float32),
        "bo": np.asarray(inputs["bo"], np.float32),
        "temp": np.ascontiguousarray(np.asarray(inputs["temperature"], np.float32).reshape(H)),
        "dfwd": dfwd,
        "dinv": dinv,
        "idn": idn,
    }
    in_maps = []
    for c in range(NCORES):
        sl = slice(c * BPC, (c + 1) * BPC)
        m = dict(shared)
        for key, name in (("queries", "xq"), ("keys", "xk"), ("values", "xv")):
            x = np.asarray(inputs[key], np.float32)[sl].reshape(T, D)
            m[name] = np.ascontiguousarray(x.T)
        in_maps.append(m)
    return in_maps


def kernel(**inputs):
    nc = _get_nc()
    in_maps = make_in_maps(inputs)
    res = run_bass_kernel_spmd(nc, in_maps, list(range(NCORES)))
    outs = [res.results[i]["out"].reshape(BPC, L, D) for i in range(NCORES)]
    return np.concatenate(outs, axis=0).astype(np.float32, copy=False)



# revision 2
# speedup vs baseline: 1.3464x; 1.3464x over previous
"""AutoCorrelationLayer kernel for 8 TRN2 NeuronCores (v2).

Math (per reference): Q/K/V projections (D=2048, H=8 heads, DH=256),
circular cross-correlation along the head dim per (b,h,l), softmax over the
correlation axis, time-delay aggregation, output projection.

v2 design:
  - All weights/activations shipped fp16 from host (no on-chip casts).
  - The forward DFT is fused into Wq/Wk on the host (q16/k16 hold spectra
    directly: per head, chunk 2h = Re(f=1..128), chunk 2h+1 = Im).  DC bin
    dropped (softmax-invariant).
  - Softmax is computed in the *transposed* (shift-major) domain:
    corr^T[s,t] from an iDFT matmul, exp with fixed shift (64/T), column
    sums via a bf16 ones-matmul (broadcast across partitions), Ln, then
    e16 = exp(corr/T - 64/T - ln(colsum)) -- no PE transposes at all.
  - bv folded into bo' = Wo@bv + bo on host (softmax rows sum to 1).
  - Per-head correlation work is interleaved into the V2/O1 projection
    matmul streams so PE never starves on DVE/ScalarE.
  - Data-parallel over batch: 4 batches/core, zero collectives.
"""

import contextlib

import numpy as np

import concourse.bass as bass
import concourse.mybir as mybir
import concourse.tile as tile_mod
from concourse.tile import TileContext
from concourse.vector_clock import ScopedClock
from concourse.bass_utils import run_bass_kernel_spmd

F32 = mybir.dt.float32
F16 = mybir.dt.float16
BF16 = mybir.dt.bfloat16
AF = mybir.ActivationFunctionType
OP = mybir.AluOpType

B, L, D, H = 32, 256, 2048, 8
DH = D // H          # 256
NCORES = 8
BPC = B // NCORES    # 4 batches per core
T = BPC * L          # 1024 tokens per core
TH = T // 2          # 512 tokens per half
EC = D // 128        # 16 feature chunks
DC = D // 128        # 16 contraction chunks
NF = 128             # retained spectrum bins (freqs 1..128)
SHIFT = 64.0         # fixed softmax stability shift (in corr units)


def _patch_tile_drain():
    """This walrus build allows at most ONE semaphore wait per instruction;
    Tile's kernel-tail drain collects one wait per live semaphore on a single
    Drain.  Split the extras onto additional drain instructions."""
    if getattr(tile_mod.TileContext, "_drain_split_patched", False):
        return

    def _drain_and_barrier(self, tick_clock, wait_clock):
        nc = self.nc
        drain_inst = nc.sync.drain()
        wait_clock.add_sem_waits(
            drain_inst.ins, ScopedClock({None: tick_clock.global_clock})
        )
        si = drain_inst.ins.sync_info
        waits = list(si.on_wait) if si is not None and si.on_wait else []
        if len(waits) > 1:
            drain_inst.ins.sync_info = mybir.SyncInfo(
                on_wait=[waits[0]], on_update=list(si.on_update or [])
            )
            for w in waits[1:]:
                extra = nc.sync.drain()
                extra.ins.sync_info = mybir.SyncInfo(on_wait=[w], on_update=[])
        nc.all_engine_barrier()
        popped = nc._tile_sem_poison_stack.pop()
        assert popped is self._sem_poison
        nc.clear_and_free_semaphores(list(self.sems.allocated().values()))
        nc.all_engine_barrier()

    tile_mod.TileContext._drain_and_barrier = _drain_and_barrier
    tile_mod.TileContext._drain_split_patched = True


def _split_multiwaits(nc):
    """Walrus in this build rejects >1 semaphore wait per instruction.  Hoist
    extra waits onto standalone EventSemaphore NOPs inserted just before the
    offending instruction on the same engine (engines execute in order)."""
    uid = [0]
    for fn in nc.m.functions:
        for bb in fn.blocks:
            il = bb.instructions
            i = 0
            while i < len(il):
                inst = il[i]
                si = inst.sync_info
                waits = list(si.on_wait) if si is not None and si.on_wait else []
                if len(waits) > 1:
                    carriers = []
                    for w in waits[:-1]:
                        uid[0] += 1
                        es = mybir.InstEventSemaphore(
                            name=f"mwsplit_{uid[0]}",
                            engine=inst.engine,
                            ins=[], outs=[],
                            sync_info=mybir.SyncInfo(on_wait=[w], on_update=[]),
                        )
                        carriers.append(es)
                    inst.sync_info = mybir.SyncInfo(
                        on_wait=[waits[-1]], on_update=list(si.on_update or [])
                    )
                    il[i:i] = carriers
                    i += len(carriers)
                i += 1


def build_kernel():
    _patch_tile_drain()
    nc = bass.Bass()

    xq = nc.declare_dram_parameter("xq", [D, T], F16, isOutput=False)  # queries^T
    xk = nc.declare_dram_parameter("xk", [D, T], F16, isOutput=False)
    xv = nc.declare_dram_parameter("xv", [D, T], F16, isOutput=False)
    wq = nc.declare_dram_parameter("wq", [D, D], F16, isOutput=False)  # (F@Wq)^T
    wk = nc.declare_dram_parameter("wk", [D, D], F16, isOutput=False)
    wv = nc.declare_dram_parameter("wv", [D, D], F16, isOutput=False)  # Wv^T
    wo = nc.declare_dram_parameter("wo", [D, D], F16, isOutput=False)  # Wo^T
    bq = nc.declare_dram_parameter("bq", [D], F32, isOutput=False)     # F@bq
    bk = nc.declare_dram_parameter("bk", [D], F32, isOutput=False)
    bo2 = nc.declare_dram_parameter("bo2", [D], F32, isOutput=False)   # Wo@bv+bo
    tmp = nc.declare_dram_parameter("temp", [H], F32, isOutput=False)
    dinv = nc.declare_dram_parameter("dinv", [2, NF, DH], F16, isOutput=False)
    out = nc.declare_dram_parameter("out", [T, D], F16, isOutput=True)

    def bcast_ap(param, n):
        return bass.AP(tensor=param, offset=0, ap=[[0, 128], [1, n]])

    with TileContext(nc) as tc:
        with contextlib.ExitStack() as ctx:
            consts = ctx.enter_context(tc.tile_pool(name="consts", bufs=1))
            persist = ctx.enter_context(tc.tile_pool(name="persist", bufs=1))

            # ---- constants ----
            Ci_sb = consts.tile([128, DH], F16, name="Ci")
            Si_sb = consts.tile([128, DH], F16, name="Si")
            nc.sync.dma_start(out=Ci_sb, in_=dinv[0, :, :])
            nc.sync.dma_start(out=Si_sb, in_=dinv[1, :, :])
            bq_sb = consts.tile([128, EC], F32)
            bk_sb = consts.tile([128, EC], F32)
            nc.sync.dma_start(out=bq_sb, in_=bq[:].rearrange("(ec p) -> p ec", p=128))
            nc.sync.dma_start(out=bk_sb, in_=bk[:].rearrange("(ec p) -> p ec", p=128))
            temp_bc = consts.tile([128, H], F32)
            nc.sync.dma_start(out=temp_bc, in_=bcast_ap(tmp, H))
            tinv = consts.tile([128, H], F32)
            nc.vector.reciprocal(tinv, temp_bc)
            nb64 = consts.tile([128, H], F32)
            nc.vector.tensor_scalar_mul(nb64, tinv, -SHIFT)
            ones_b16 = consts.tile([128, 128], BF16, name="ones")
            nc.vector.memset(ones_b16[:], 1.0)

            q16 = persist.tile([128, EC, T], F16, name="q16")
            k16 = persist.tile([128, EC, T], F16, name="k16")
            v16 = persist.tile([128, T // 128, D], F16, name="v16")
            outf16 = persist.tile([128, EC, T], F16, name="outf16")

            # ---------------- per-head correlation block -----------------
            def emit_head(h, half, hp, ep, psD, psC, psO):
                t0 = half * TH
                qr = q16[:, 2 * h, t0:t0 + TH]
                qi = q16[:, 2 * h + 1, t0:t0 + TH]
                kr = k16[:, 2 * h, t0:t0 + TH]
                ki = k16[:, 2 * h + 1, t0:t0 + TH]
                m1 = hp.tile([128, TH], F16, tag="m1")
                m2 = hp.tile([128, TH], F16, tag="m2")
                m3 = hp.tile([128, TH], F16, tag="m3")
                m4 = hp.tile([128, TH], F16, tag="m4")
                pr = hp.tile([128, TH], F16, tag="pr")
                pi = hp.tile([128, TH], F16, tag="pi")
                # P = Q * conj(K) (elementwise over freq x token)
                nc.vector.tensor_mul(m1, qr, kr)
                nc.vector.tensor_mul(m2, qi, ki)
                nc.vector.tensor_add(pr, m1, m2)
                nc.vector.tensor_mul(m3, qi, kr)
                nc.vector.tensor_mul(m4, qr, ki)
                nc.vector.tensor_sub(pi, m3, m4)
                # iDFT straight to corr^T[s, t] (2 shift-chunks)
                psTs, ebs = [], []
                for sck in range(2):
                    ps = psD.tile([128, TH], F32, tag="psT")
                    nc.tensor.matmul(ps, Ci_sb[:, sck * 128:(sck + 1) * 128],
                                     pr, start=True, stop=False)
                    nc.tensor.matmul(ps, Si_sb[:, sck * 128:(sck + 1) * 128],
                                     pi, start=False, stop=True)
                    eb = ep.tile([128, TH], BF16, tag="eb")
                    nc.scalar.activation(eb, ps, AF.Exp,
                                         bias=nb64[:, h:h + 1],
                                         scale=tinv[:, h:h + 1])
                    psTs.append(ps)
                    ebs.append(eb)
                # column sums broadcast across partitions via ones-matmul
                pcs = psC.tile([128, TH], F32, tag="pcs")
                nc.tensor.matmul(pcs, ones_b16[:], ebs[0], start=True, stop=False)
                nc.tensor.matmul(pcs, ones_b16[:], ebs[1], start=False, stop=True)
                lncs = ep.tile([128, TH], F32, tag="lncs")
                nc.scalar.activation(lncs, pcs, AF.Ln)
                e16s = []
                for sck in range(2):
                    m32 = ep.tile([128, TH], F32, tag="m32")
                    nc.vector.scalar_tensor_tensor(
                        m32, psTs[sck], tinv[:, h:h + 1], lncs,
                        OP.mult, OP.subtract)
                    e16 = ep.tile([128, TH], F16, tag="e16")
                    nc.scalar.activation(e16, m32, AF.Exp, bias=nb64[:, h:h + 1])
                    e16s.append(e16)
                # TDA: outf[i, t] = sum_s V[s,i] * E[s,t], per local batch
                for b in range(2):
                    for ic in range(2):
                        pso = psO.tile([128, L], F32, tag=f"o{ic}")
                        for sc in range(2):
                            nc.tensor.matmul(
                                pso,
                                v16[:, half * 4 + b * 2 + sc,
                                    h * DH + ic * 128:h * DH + (ic + 1) * 128],
                                e16s[sc][:, b * L:(b + 1) * L],
                                start=(sc == 0), stop=(sc == 1))
                        dst = outf16[:, 2 * h + ic, t0 + b * L:t0 + (b + 1) * L]
                        if (b + ic) % 2 == 0:
                            nc.scalar.activation(dst, pso, AF.Copy)
                        else:
                            nc.vector.tensor_copy(dst, pso)

            # ---------------- Q/K spectral projections -------------------
            with tc.tile_pool(name="xqk", bufs=2) as xpool, \
                 tc.tile_pool(name="wqk", bufs=2) as wpool, \
                 tc.tile_pool(name="psP", bufs=8, space="PSUM") as psP:
                for (xpar, wpar, bsb, dst16) in ((xq, wq, bq_sb, q16),
                                                 (xk, wk, bk_sb, k16)):
                    for tn in range(2):
                        xh = xpool.tile([128, DC, TH], F16, tag="xh")
                        for dh in range(2):
                            nc.sync.dma_start(
                                out=xh[:, dh * 8:(dh + 1) * 8, :],
                                in_=xpar[dh * 1024:(dh + 1) * 1024,
                                         tn * TH:(tn + 1) * TH]
                                .rearrange("(c p) t -> p c t", p=128))
                        for g in range(4):
                            wt = wpool.tile([128, DC, TH], F16, tag="wt")
                            for dh in range(2):
                                nc.sync.dma_start(
                                    out=wt[:, dh * 8:(dh + 1) * 8, :],
                                    in_=wpar[dh * 1024:(dh + 1) * 1024,
                                             g * 512:(g + 1) * 512]
                                    .rearrange("(c p) e -> p c e", p=128))
                            pss = [psP.tile([128, TH], F32, tag="psP",
                                            name=f"psp_{tn}_{g}_{j}")
                                   for j in range(4)]
                            for dc in range(DC):
                                for j in range(4):
                                    nc.tensor.matmul(
                                        pss[j], wt[:, dc, j * 128:(j + 1) * 128],
                                        xh[:, dc, :],
                                        start=(dc == 0), stop=(dc == DC - 1))
                            for j in range(4):
                                ec = g * 4 + j
                                dst = dst16[:, ec, tn * TH:(tn + 1) * TH]
                                if j % 2 == 0:
                                    nc.scalar.activation(dst, pss[j], AF.Identity,
                                                         bias=bsb[:, ec:ec + 1])
                                else:
                                    nc.vector.tensor_scalar_add(dst, pss[j],
                                                                bsb[:, ec:ec + 1])

            # ------------- V projection + heads, O projection ------------
            with tc.tile_pool(name="hp", bufs=1) as hp, \
                 tc.tile_pool(name="ep", bufs=2) as ep, \
                 tc.tile_pool(name="psD", bufs=2, space="PSUM") as psD, \
                 tc.tile_pool(name="psC", bufs=1, space="PSUM") as psC, \
                 tc.tile_pool(name="psO", bufs=1, space="PSUM") as psO:

                with tc.tile_pool(name="xv", bufs=1) as xvpool, \
                     tc.tile_pool(name="wvp", bufs=2) as wvpool, \
                     tc.tile_pool(name="psV", bufs=3, space="PSUM") as psV:
                    for half in range(2):
                        t0 = half * TH
                        xh = xvpool.tile([128, DC, TH], F16, tag="xvh")
                        for dh in range(2):
                            nc.sync.dma_start(
                                out=xh[:, dh * 8:(dh + 1) * 8, :],
                                in_=xv[dh * 1024:(dh + 1) * 1024, t0:t0 + TH]
                                .rearrange("(c p) t -> p c t", p=128))
                        blk = 0
                        for g in range(4):
                            wt = wvpool.tile([128, DC, TH], F16, tag="wvt")
                            for dh in range(2):
                                nc.sync.dma_start(
                                    out=wt[:, dh * 8:(dh + 1) * 8, :],
                                    in_=wv[dh * 1024:(dh + 1) * 1024,
                                           g * 512:(g + 1) * 512]
                                    .rearrange("(c p) e -> p c e", p=128))
                            for tckg in range(2):
                                psv = [psV.tile([128, TH], F32, tag="psV",
                                                name=f"psv_{half}_{g}_{tckg}_{i}")
                                       for i in range(2)]
                                for dc in range(DC):
                                    for i in range(2):
                                        tl = tckg * 2 + i
                                        nc.tensor.matmul(
                                            psv[i],
                                            xh[:, dc, tl * 128:(tl + 1) * 128],
                                            wt[:, dc, :],
                                            start=(dc == 0), stop=(dc == DC - 1))
                                for i in range(2):
                                    tck = half * 4 + tckg * 2 + i
                                    dst = v16[:, tck, g * 512:(g + 1) * 512]
                                    if i == 0:
                                        nc.scalar.activation(dst, psv[i], AF.Copy)
                                    else:
                                        nc.vector.tensor_copy(dst, psv[i])
                                if half == 1:
                                    # interleave half-0 heads into V2 stream
                                    emit_head(blk, 0, hp, ep, psD, psC, psO)
                                blk += 1

                # ---- output projection (+ interleaved half-1 heads) ----
                with tc.tile_pool(name="wop", bufs=2) as wopool, \
                     tc.tile_pool(name="ypool", bufs=4) as ypool, \
                     tc.tile_pool(name="psY", bufs=3, space="PSUM") as psY:
                    bo_bc = wopool.tile([128, D], F32, tag="bo", bufs=1)
                    nc.sync.dma_start(out=bo_bc, in_=bcast_ap(bo2, D))
                    blk = 0
                    for tgrp in range(2):          # token halves of O-proj
                        for ocg in range(4):
                            wot = wopool.tile([128, EC, TH], F16, tag="wot")
                            for dh in range(2):
                                nc.sync.dma_start(
                                    out=wot[:, dh * 8:(dh + 1) * 8, :],
                                    in_=wo[dh * 1024:(dh + 1) * 1024,
                                           ocg * 512:(ocg + 1) * 512]
                                    .rearrange("(c p) e -> p c e", p=128))
                            for tcl in range(4):
                                tck = tgrp * 4 + tcl
                                psy = psY.tile([128, TH], F32, tag="psY",
                                               name=f"psy_{tck}_{ocg}")
                                for ec in range(EC):
                                    nc.tensor.matmul(
                                        psy,
                                        outf16[:, ec, tck * 128:(tck + 1) * 128],
                                        wot[:, ec, :],
                                        start=(ec == 0), stop=(ec == EC - 1))
                                yt = ypool.tile([128, TH], F16, tag="yt")
                                nc.vector.tensor_add(
                                    yt, psy, bo_bc[:, ocg * 512:(ocg + 1) * 512])
                                nc.sync.dma_start(
                                    out=out[tck * 128:(tck + 1) * 128,
                                            ocg * 512:(ocg + 1) * 512],
                                    in_=yt)
                                if tgrp == 0 and blk % 2 == 0:
                                    # interleave half-1 heads into O1 stream
                                    emit_head(blk // 2, 1, hp, ep, psD, psC, psO)
                                blk += 1
    _split_multiwaits(nc)
    return nc


_NC_CACHE = None


def _get_nc():
    global _NC_CACHE
    if _NC_CACHE is None:
        _NC_CACHE = build_kernel()
    return _NC_CACHE


def _dft_consts():
    m = np.arange(DH, dtype=np.float64)
    f = np.arange(1, NF + 1, dtype=np.float64)   # freqs 1..128 (DC dropped)
    ang_f = 2.0 * np.pi * np.outer(m, f) / DH
    C = np.cos(ang_f)            # [m, NF]
    S = -np.sin(ang_f)
    n = np.arange(DH, dtype=np.float64)
    w = np.where(f < NF, 2.0, 1.0)[:, None]      # conj-symmetry weights
    ang_i = 2.0 * np.pi * np.outer(f, n) / DH
    Ci = w * np.cos(ang_i) / DH  # [NF, n]
    Si = -w * np.sin(ang_i) / DH
    return C, S, Ci, Si


def make_in_maps(inputs):
    C, S, Ci, Si = _dft_consts()
    dinv = np.stack([Ci, Si]).astype(np.float16)

    def fuse_dft(W, b):
        """Per head: rows h*256..h*256+127 = Re spectrum, +128.. = Im."""
        W = np.asarray(W, np.float64)
        b = np.asarray(b, np.float64)
        W2 = np.empty_like(W)
        b2 = np.empty_like(b)
        for h in range(H):
            blkW = W[h * DH:(h + 1) * DH, :]     # [m, d]
            blkb = b[h * DH:(h + 1) * DH]
            W2[h * DH:h * DH + NF, :] = C.T @ blkW
            W2[h * DH + NF:(h + 1) * DH, :] = S.T @ blkW
            b2[h * DH:h * DH + NF] = C.T @ blkb
            b2[h * DH + NF:(h + 1) * DH] = S.T @ blkb
        return W2, b2

    Wq2, bq2 = fuse_dft(inputs["Wq"], inputs["bq"])
    Wk2, bk2 = fuse_dft(inputs["Wk"], inputs["bk"])
    Wo = np.asarray(inputs["Wo"], np.float64)
    bo2 = Wo @ np.asarray(inputs["bv"], np.float64) + np.asarray(
        inputs["bo"], np.float64)

    shared = {
        "wq": np.ascontiguousarray(Wq2.T).astype(np.float16),
        "wk": np.ascontiguousarray(Wk2.T).astype(np.float16),
        "wv": np.ascontiguousarray(np.asarray(inputs["Wv"]).T).astype(np.float16),
        "wo": np.ascontiguousarray(Wo.T).astype(np.float16),
        "bq": bq2.astype(np.float32),
        "bk": bk2.astype(np.float32),
        "bo2": bo2.astype(np.float32),
        "temp": np.ascontiguousarray(
            np.asarray(inputs["temperature"], np.float32).reshape(H)),
        "dinv": dinv,
    }
    in_maps = []
    for c in range(NCORES):
        sl = slice(c * BPC, (c + 1) * BPC)
        m = dict(shared)
        for key, name in (("queries", "xq"), ("keys", "xk"), ("values", "xv")):
            x = np.asarray(inputs[key], np.float32)[sl].reshape(T, D)
            m[name] = np.ascontiguousarray(x.T).astype(np.float16)
        in_maps.append(m)
    return in_maps


def kernel(**inputs):
    nc = _get_nc()
    in_maps = make_in_maps(inputs)
    res = run_bass_kernel_spmd(nc, in_maps, list(range(NCORES)))
    outs = [res.results[i]["out"].astype(np.float32).reshape(BPC, L, D)
            for i in range(NCORES)]
    return np.concatenate(outs, axis=0)
